# revision 23
# baseline (speedup 1.0000x reference)
"""Trainium2 Bass kernel for nn_BranchFusSSM (VMamba-style cross-scan selective SSM).

Sharding: 8 cores = (batch b in {0,1}) x (scan direction k in {0..3}).

Launch 1 (per core): lane layout = 12 tiles of 128 lanes, lane (d, n) with
d = 8*t + p//16, n = p%16.  Per 1024-col block: input projections emit
delta-pre [96], u [96], and B/C directly in lane-replicated form [128]
(replication folded into the projection weights on host).  delta =
softplus (1 ACT op).  Per tile: delta-rep via PE selector matmul -> exp
(ACT, per-lane A scale); w = delta*u replicated via a 0-stride SBUF->SBUF
DMA (idle DMA engines); b = w*B and sty = h*C run as bf16 2x DVE ops;
tensor_tensor_scan split across DVE/Pool; n-contraction on PE into PSUM.
Scan-side elementwise ops process 2 blocks (2048 cols) per instruction.

Launch 2 (8 cores = (b, quarter-of-L)): sum 4 direction contributions
(bf16 2x adds), add (sum_k D_k) * u (u recomputed from y via PE),
LayerNorm over channels via PE ones-matmuls + row-stats, output
projection with gamma folded; the -mu*rstd*Wout@gamma + Wout@beta terms
enter the projection PSUM via a rank-2 outer-product matmul.

Host work between launches is pure permutation/slicing (numpy).
"""

import sys

if "/opt/trn_rl_repo" not in sys.path:
    sys.path.insert(0, "/opt/trn_rl_repo")

import numpy as np
import ml_dtypes

import concourse.bacc as bacc
import concourse.mybir as mybir
import concourse.hw_specs as _hw_specs
from concourse.tile import TileContext
from concourse import bass_utils

# Force every activation onto the one table containing exp+ln+copy+square
# (the only functions this kernel uses), so the scheduler never inserts ACT
# table loads (~2.7us per switch).
_ORIG_GAT = _hw_specs.get_activation_tables
_ONE_TABLE = "natural_log_exp_and_others"


def _gat_single(arch):
    full = _ORIG_GAT(arch)
    return {name: (funcs if name == _ONE_TABLE else set())
            for name, funcs in full.items()}


bacc.get_activation_tables = _gat_single

# problem constants (hardcoded per contract)
B = 2
DM = 96          # d_model
DI = 96          # d_inner
H = W = 128
L = H * W        # 16384
N = 16           # d_state
R = 6            # dt_rank
K = 4            # directions
LN_EPS = 1e-5

NT = 12          # lane tiles: 12 x (8 d-channels x 16 n) = 1536 states
DPT = 8          # d-channels per lane tile
BLK = 1024       # PSUM-side block
GRP = 2048       # SBUF-side group (2 blocks)
NGRP = L // GRP  # 8
F32 = mybir.dt.float32
F32R = mybir.dt.float32r
BF16 = mybir.dt.bfloat16

N_CORES = 8
ALU = mybir.AluOpType
AF = mybir.ActivationFunctionType

# tiles whose sb / sty muls run on Pool (rest on DVE as bf16-2x TT); the
# scan and TensorScalarPtr ops are DVE-only (illegal opcodes on Pool).
SB_POOL = 5
STY_POOL = 5


# ---------------------------------------------------------------------------
# host-side helpers
# ---------------------------------------------------------------------------

def _perm(t2d: np.ndarray, k: int) -> np.ndarray:
    """[C, H, W] image -> [C, L] sequence in direction-k scan order."""
    c = t2d.shape[0]
    if k == 0:
        return np.ascontiguousarray(t2d.reshape(c, L))
    if k == 1:
        return np.ascontiguousarray(t2d.transpose(0, 2, 1).reshape(c, L))
    if k == 2:
        return np.ascontiguousarray(t2d.reshape(c, L)[:, ::-1])
    return np.ascontiguousarray(t2d.transpose(0, 2, 1).reshape(c, L)[:, ::-1])


def _unperm(seq: np.ndarray, k: int) -> np.ndarray:
    """inverse of _perm: direction-k ordered [C, L] -> row-major [C, L]."""
    c = seq.shape[0]
    if k == 0:
        return seq
    if k == 1:
        return np.ascontiguousarray(seq.reshape(c, W, H).transpose(0, 2, 1).reshape(c, L))
    if k == 2:
        return np.ascontiguousarray(seq[:, ::-1])
    rev = seq[:, ::-1]
    return np.ascontiguousarray(rev.reshape(c, W, H).transpose(0, 2, 1).reshape(c, L))


def _lane_maps():
    """d(p) = DPT*t + p//N, n(p) = p%N for lane p of tile t."""
    p = np.arange(128)
    return p // N, p % N


def _selectors():
    pd, pn = _lane_maps()
    wsel = np.zeros((DI, NT * 128), np.float32)   # replicate d-rows into lanes
    ysel = np.zeros((128, NT * DI), np.float32)   # reduce lanes back to d-rows
    for t in range(NT):
        for p in range(128):
            d = DPT * t + pd[p]
            wsel[d, t * 128 + p] = 1.0
            ysel[p, t * DI + d] = 1.0
    return wsel, ysel


# ---------------------------------------------------------------------------
# launch 1: per-direction selective scan
# ---------------------------------------------------------------------------

def build_scan_program():
    nc = bacc.Bacc("TRN2", target_bir_lowering=False, debug=False)

    xk = nc.dram_tensor("xk", [DM, L], F32R, kind="ExternalInput")
    yk = nc.dram_tensor("yk", [DM, L], F32R, kind="ExternalInput")
    wdT = nc.dram_tensor("wdT", [DM, DI], F32R, kind="ExternalInput")
    wuT = nc.dram_tensor("wuT", [DM, DI], F32R, kind="ExternalInput")
    wBrepT = nc.dram_tensor("wBrepT", [DM, 128], F32R, kind="ExternalInput")
    wCrepT = nc.dram_tensor("wCrepT", [DM, 128], F32R, kind="ExternalInput")
    dtbias = nc.dram_tensor("dtbias", [DI, 1], F32, kind="ExternalInput")
    ascale = nc.dram_tensor("ascale", [128, NT], F32, kind="ExternalInput")
    wsel = nc.dram_tensor("wsel", [DI, NT * 128], BF16, kind="ExternalInput")
    ysel = nc.dram_tensor("ysel", [128, NT * DI], BF16, kind="ExternalInput")
    yc = nc.dram_tensor("yc", [DI, L], BF16, kind="ExternalOutput")

    with TileContext(nc) as tc:
        with (
            tc.tile_pool(name="smalls", bufs=1) as smalls,
            tc.tile_pool(name="io", bufs=3) as io,
            tc.tile_pool(name="stage", bufs=2) as stage,
            tc.tile_pool(name="lanes", bufs=3) as lanes,
            tc.tile_pool(name="psum", bufs=1, space="PSUM") as psum,
        ):
            s_wdT = smalls.tile([DM, DI], F32R, tag="wdT")
            s_wuT = smalls.tile([DM, DI], F32R, tag="wuT")
            s_wB = smalls.tile([DM, 128], F32R, tag="wB")
            s_wC = smalls.tile([DM, 128], F32R, tag="wC")
            s_bias = smalls.tile([DI, 1], F32, tag="bias")
            s_asc = smalls.tile([128, NT], F32, tag="asc")
            s_wsel = smalls.tile([DI, NT * 128], BF16, tag="wsel")
            s_ysel = smalls.tile([128, NT * DI], BF16, tag="ysel")
            nc.sync.dma_start(s_wdT[:], wdT.ap())
            nc.sync.dma_start(s_wuT[:], wuT.ap())
            nc.sync.dma_start(s_wB[:], wBrepT.ap())
            nc.sync.dma_start(s_wC[:], wCrepT.ap())
            nc.sync.dma_start(s_bias[:], dtbias.ap())
            nc.sync.dma_start(s_asc[:], ascale.ap())
            nc.sync.dma_start(s_wsel[:], wsel.ap())
            nc.sync.dma_start(s_ysel[:], ysel.ap())

            htails = [smalls.tile([128, 1], F32, tag=f"ht{t}", name=f"ht{t}")
                      for t in range(NT)]

            def h_loop(g):
                sdb2 = stage.tile([DI, GRP], BF16, tag="sdb2")
                sw2 = stage.tile([DI, GRP], BF16, tag="sw2")
                sB2 = stage.tile([128, GRP], BF16, tag="sB2")
                sC2 = stage.tile([128, GRP], BF16, tag="sC2")
                for h in range(2):
                    lo = g * GRP + h * BLK
                    hs = slice(h * BLK, (h + 1) * BLK)
                    xkb = io.tile([DM, BLK], F32R, tag="xkb")
                    ykb = io.tile([DM, BLK], F32R, tag="ykb")
                    nc.sync.dma_start(xkb[:], xk.ap()[:, lo:lo + BLK])
                    nc.sync.dma_start(ykb[:], yk.ap()[:, lo:lo + BLK])

                    pd_ = psum.tile([128, BLK], F32, tag="ps", bufs=3)
                    for c0 in range(0, BLK, 512):
                        nc.tensor.matmul(pd_[0:DI, c0:c0 + 512], s_wdT[:],
                                         xkb[:, c0:c0 + 512],
                                         start=True, stop=True)
                    # delta = softplus(z) = ln(exp(z + bias) + 1)
                    sez = io.tile([DI, BLK], BF16, tag="sez")
                    nc.scalar.activation(sez[:], pd_[0:DI, :], AF.Exp,
                                         bias=s_bias[:], scale=1.0)
                    nc.scalar.activation(sdb2[:, hs], sez[:], AF.Ln, bias=1.0)

                    pu = psum.tile([128, BLK], F32, tag="ps", bufs=3)
                    for c0 in range(0, BLK, 512):
                        nc.tensor.matmul(pu[0:DI, c0:c0 + 512], s_wuT[:],
                                         ykb[:, c0:c0 + 512],
                                         start=True, stop=True)
                    nc.vector.tensor_mul(sw2[:, hs], sdb2[:, hs], pu[0:DI, :])

                    pB = psum.tile([128, BLK], F32, tag="ps", bufs=3)
                    for c0 in range(0, BLK, 512):
                        nc.tensor.matmul(pB[:, c0:c0 + 512], s_wB[:],
                                         xkb[:, c0:c0 + 512],
                                         start=True, stop=True)
                    nc.scalar.copy(sB2[:, hs], pB[:])

                    pC = psum.tile([128, BLK], F32, tag="ps", bufs=3)
                    for c0 in range(0, BLK, 512):
                        nc.tensor.matmul(pC[:, c0:c0 + 512], s_wC[:],
                                         xkb[:, c0:c0 + 512],
                                         start=True, stop=True)
                    nc.scalar.copy(sC2[:, hs], pC[:])
                return sdb2, sw2, sB2, sC2

            def tile_loop(g, sdb2, sw2, sB2, sC2):
                stys = []
                for t in range(NT):
                    # w replicated into lanes via 0-stride SBUF->SBUF DMA
                    swrep = lanes.tile([128, GRP], BF16, tag="swrep", bufs=2)
                    nc.sync.dma_start(
                        swrep[:],
                        sw2[DPT * t:DPT * t + DPT, :].unsqueeze(1)
                            .broadcast_to([DPT, N, GRP]))

                    sa2 = lanes.tile([128, GRP], F32, tag="sa", bufs=2)
                    for h in range(2):
                        hs = slice(h * BLK, (h + 1) * BLK)
                        pa = psum.tile([128, BLK], F32, tag="ps", bufs=3)
                        for c0 in range(0, BLK, 512):
                            nc.tensor.matmul(
                                pa[:, c0:c0 + 512],
                                s_wsel[:, t * 128:(t + 1) * 128],
                                sdb2[:, h * BLK + c0:h * BLK + c0 + 512],
                                start=True, stop=True)
                        nc.scalar.activation(sa2[:, hs], pa[:], AF.Exp,
                                             scale=s_asc[:, t:t + 1])

                    sb2 = lanes.tile([128, GRP], BF16, tag="sb", bufs=2)
                    if t >= NT - SB_POOL:
                        nc.gpsimd.tensor_mul(sb2[:, 0:BLK], swrep[:, 0:BLK],
                                             sB2[:, 0:BLK])
                        nc.gpsimd.tensor_mul(sb2[:, BLK:GRP], swrep[:, BLK:GRP],
                                             sB2[:, BLK:GRP])
                    else:
                        nc.vector.tensor_mul(sb2[:], swrep[:], sB2[:])

                    sh2 = lanes.tile([128, GRP], BF16, tag="sh", bufs=2)
                    init = 0.0 if g == 0 else htails[t][:]
                    nc.vector.tensor_tensor_scan(sh2[:], sa2[:], sb2[:], init,
                                                 op0=ALU.mult, op1=ALU.add)
                    nc.vector.tensor_scalar_add(htails[t][:],
                                                sh2[:, GRP - 1:GRP], 0.0)

                    sty2 = lanes.tile([128, GRP], BF16, tag=f"sty{t}",
                                      bufs=1, name=f"sty{t}")
                    if t < STY_POOL:
                        nc.gpsimd.tensor_mul(sty2[:, 0:BLK], sh2[:, 0:BLK],
                                             sC2[:, 0:BLK])
                        nc.gpsimd.tensor_mul(sty2[:, BLK:GRP], sh2[:, BLK:GRP],
                                             sC2[:, BLK:GRP])
                    else:
                        nc.vector.tensor_mul(sty2[:], sh2[:], sC2[:])
                    stys.append(sty2)
                return stys

            def y_batch(g, stys):
                # y-contraction issued after the NEXT group's projections so
                # its operand waits never stall PE's in-order queue
                for h in range(2):
                    lo = g * GRP + h * BLK
                    yp = psum.tile([DI, BLK], F32, tag="yacc", bufs=1,
                                   name=f"yp{g}_{h}")
                    for t in range(NT):
                        for c0 in range(0, BLK, 512):
                            nc.tensor.matmul(
                                yp[:, c0:c0 + 512],
                                s_ysel[:, t * DI:(t + 1) * DI],
                                stys[t][:, h * BLK + c0:h * BLK + c0 + 512],
                                start=(t == 0), stop=(t == NT - 1))
                    syc = io.tile([DI, BLK], BF16, tag="syc")
                    nc.scalar.copy(syc[:], yp[:])
                    nc.scalar.dma_start(yc.ap()[:, lo:lo + BLK], syc[:])

            stage_t = h_loop(0)
            for g in range(NGRP):
                stys = tile_loop(g, *stage_t)
                if g + 1 < NGRP:
                    stage_t = h_loop(g + 1)
                y_batch(g, stys)

    nc.compile()
    return nc


# ---------------------------------------------------------------------------
# launch 2: merge 4 directions + D*u + LayerNorm + output projection
# ---------------------------------------------------------------------------

L2 = L // 4      # positions per core: 4096
C2 = 1024        # processing chunk


def build_merge_program():
    nc = bacc.Bacc("TRN2", target_bir_lowering=False, debug=False)

    cin = [nc.dram_tensor(f"c{i}", [DI, L2], F32R, kind="ExternalInput")
           for i in range(K)]
    ykq = nc.dram_tensor("ykq", [DM, L2], BF16, kind="ExternalInput")
    wgT = nc.dram_tensor("wgT", [DI, DM], F32R, kind="ExternalInput")
    wuT = nc.dram_tensor("wuT", [DM, DI], BF16, kind="ExternalInput")
    vg1 = nc.dram_tensor("vg1", [1, DM], F32R, kind="ExternalInput")
    vb1 = nc.dram_tensor("vb1", [1, DM], F32R, kind="ExternalInput")
    dvs = nc.dram_tensor("dvs", [DI, 1], F32, kind="ExternalInput")
    onesM = nc.dram_tensor("onesM", [DI, 1], F32R, kind="ExternalInput")
    ones1 = nc.dram_tensor("ones1", [1, DI], F32R, kind="ExternalInput")
    epsv = nc.dram_tensor("epsv", [1, 1], F32, kind="ExternalInput")
    out2 = nc.dram_tensor("out2", [DM, L2], F32, kind="ExternalOutput")

    with TileContext(nc) as tc:
        with (
            tc.tile_pool(name="smalls", bufs=1) as smalls,
            tc.tile_pool(name="work", bufs=2) as work,
            tc.tile_pool(name="psum", bufs=1, space="PSUM") as psum,
        ):
            s_wgT = smalls.tile([DI, DM], F32R, tag="wgT")
            s_wuT = smalls.tile([DM, DI], BF16, tag="wuT")
            s_vg1 = smalls.tile([1, DM], F32R, tag="vg1")
            s_vb1 = smalls.tile([1, DM], F32R, tag="vb1")
            s_dvs = smalls.tile([DI, 1], F32, tag="dvs")
            s_ones = smalls.tile([DI, 1], F32R, tag="ones")
            s_ones1 = smalls.tile([1, DI], F32R, tag="ones1")
            s_eps = smalls.tile([1, 1], F32, tag="eps")
            nc.sync.dma_start(s_wgT[:], wgT.ap())
            nc.sync.dma_start(s_wuT[:], wuT.ap())
            nc.sync.dma_start(s_vg1[:], vg1.ap())
            nc.sync.dma_start(s_vb1[:], vb1.ap())
            nc.sync.dma_start(s_dvs[:], dvs.ap())
            nc.sync.dma_start(s_ones[:], onesM.ap())
            nc.sync.dma_start(s_ones1[:], ones1.ap())
            nc.sync.dma_start(s_eps[:], epsv.ap())

            for j in range(L2 // C2):
                sl = slice(j * C2, (j + 1) * C2)
                cb = []
                for i in range(K):
                    t = work.tile([DI, C2], F32R, tag=f"cin{i}", name=f"cin{i}")
                    nc.sync.dma_start(t[:], cin[i].ap()[:, sl])
                    cb.append(t)
                ykb = work.tile([DM, C2], BF16, tag="ykb")
                nc.sync.dma_start(ykb[:], ykq.ap()[:, sl])

                t01 = work.tile([DI, C2], F32, tag="t01")
                t23 = work.tile([DI, C2], F32, tag="t23")
                s4b = work.tile([DI, C2], F32, tag="s4b")
                nc.vector.tensor_add(t01[:], cb[0][:], cb[1][:])
                nc.vector.tensor_add(t23[:], cb[2][:], cb[3][:])
                nc.vector.tensor_add(s4b[:], t01[:], t23[:])

                # u = Wy @ y (row-major), s4 = s4b + (sum_k D_k) * u
                pu = psum.tile([DI, C2], F32, tag="m96", bufs=1)
                for c0 in range(0, C2, 512):
                    nc.tensor.matmul(pu[:, c0:c0 + 512], s_wuT[:],
                                     ykb[:, c0:c0 + 512],
                                     start=True, stop=True)
                s4 = work.tile([DI, C2], F32R, tag="s4")
                nc.vector.scalar_tensor_tensor(s4[:], pu[:], s_dvs[:], s4b[:],
                                               op0=ALU.mult, op1=ALU.add)

                ssq = work.tile([DI, C2], F32R, tag="ssq")
                nc.scalar.activation(ssq[:], s4[:], AF.Square)

                pmu = psum.tile([1, C2], F32, tag="pmu", bufs=1)
                psq = psum.tile([1, C2], F32, tag="psq", bufs=1)
                for c0 in range(0, C2, 512):
                    nc.tensor.matmul(pmu[:, c0:c0 + 512], s_ones[:],
                                     s4[:, c0:c0 + 512],
                                     start=True, stop=True)
                    nc.tensor.matmul(psq[:, c0:c0 + 512], s_ones[:],
                                     ssq[:, c0:c0 + 512],
                                     start=True, stop=True)
                smusq = work.tile([1, C2], F32, tag="smusq")
                nc.scalar.activation(smusq[:], pmu[:], AF.Square)
                svar = work.tile([1, C2], F32, tag="svar")
                nc.vector.tensor_sub(svar[:], psq[:], smusq[:])
                # rsqrt(var+eps) = exp(-0.5 * ln(var+eps))
                slnv = work.tile([1, C2], F32, tag="slnv")
                nc.scalar.activation(slnv[:], svar[:], AF.Ln, bias=s_eps[:])
                srstd = work.tile([1, C2], F32R, tag="srstd")
                nc.scalar.activation(srstd[:], slnv[:], AF.Exp, scale=-0.5)
                srcp = work.tile([1, C2], F32R, tag="srcp")
                nc.scalar.activation(srcp[:], slnv[:], AF.Exp, scale=0.5)
                smu = work.tile([1, C2], F32R, tag="smu")
                nc.scalar.copy(smu[:], pmu[:])

                # B1 = broadcast rstd over 96 channel rows
                pB1 = psum.tile([DI, C2], F32, tag="m96", bufs=1)
                for c0 in range(0, C2, 512):
                    nc.tensor.matmul(pB1[:, c0:c0 + 512], s_ones1[:],
                                     srstd[:, c0:c0 + 512],
                                     start=True, stop=True)
                sB1 = work.tile([DM, C2], BF16, tag="sB1")
                nc.scalar.copy(sB1[:], pB1[:])

                # pq = Wout·gamma @ s4 + vgneg (x) mu + vbeta (x) (1/rstd);
                # the final multiply by the broadcast rstd then yields
                # rstd*pq + vgneg*(mu*rstd) + vbeta.
                pq = psum.tile([DM, C2], F32, tag="pq", bufs=1)
                for c0 in range(0, C2, 512):
                    nc.tensor.matmul(pq[:, c0:c0 + 512], s_wgT[:],
                                     s4[:, c0:c0 + 512],
                                     start=True, stop=False)
                    nc.tensor.matmul(pq[:, c0:c0 + 512], s_vg1[:],
                                     smu[:, c0:c0 + 512],
                                     start=False, stop=False)
                    nc.tensor.matmul(pq[:, c0:c0 + 512], s_vb1[:],
                                     srcp[:, c0:c0 + 512],
                                     start=False, stop=True)
                so = work.tile([DM, C2], F32, tag="so")
                nc.vector.tensor_mul(so[:], pq[:], sB1[:])
                nc.sync.dma_start(out2.ap()[:, sl], so[:])

    nc.compile()
    return nc


# ---------------------------------------------------------------------------
# host orchestration
# ---------------------------------------------------------------------------

_CACHE: dict = {}


def _programs():
    if "p1" not in _CACHE:
        _CACHE["p1"] = build_scan_program()
        _CACHE["p2"] = build_merge_program()
    return _CACHE["p1"], _CACHE["p2"]


def kernel(x, y, Wx, Wy, x_proj_weight, dt_projs_weight, dt_projs_bias,
           A_logs, Ds, ln_gamma, ln_beta, Wout):
    x = np.asarray(x, np.float32)
    y = np.asarray(y, np.float32)
    f8 = lambda a: np.asarray(a, np.float64)

    wsel_np, ysel_np = _selectors()
    pd, pn = _lane_maps()
    A = -np.exp(f8(A_logs)).reshape(K, DI, N)
    Dv = f8(Ds).reshape(K, DI)

    nc1, nc2 = _programs()

    in_maps1 = []
    for core in range(N_CORES):
        b, k = core // K, core % K
        Wd = (f8(dt_projs_weight)[k] @ f8(x_proj_weight)[k][:R] @ f8(Wx))
        WB = f8(x_proj_weight)[k][R:R + N] @ f8(Wx)     # [N, DM]
        WC = f8(x_proj_weight)[k][R + N:] @ f8(Wx)

        asc = np.empty((128, NT), np.float32)
        for t in range(NT):
            asc[:, t] = A[k][DPT * t + pd, pn]

        in_maps1.append(dict(
            xk=_perm(x[b], k),
            yk=_perm(y[b], k),
            wdT=np.ascontiguousarray(Wd.T.astype(np.float32)),
            wuT=np.ascontiguousarray(f8(Wy).T.astype(np.float32)),
            wBrepT=np.ascontiguousarray(WB[pn].T.astype(np.float32)),
            wCrepT=np.ascontiguousarray(WC[pn].T.astype(np.float32)),
            dtbias=np.asarray(dt_projs_bias, np.float32)[k].reshape(DI, 1),
            ascale=asc,
            wsel=wsel_np.astype(ml_dtypes.bfloat16),
            ysel=ysel_np.astype(ml_dtypes.bfloat16),
        ))

    res1 = bass_utils.run_bass_kernel_spmd(nc1, in_maps1,
                                           core_ids=list(range(N_CORES)))
    _CACHE["res1"] = res1

    # un-permute each direction's contribution back to row-major order
    contrib = np.empty((B, K, DI, L), np.float32)
    for core in range(N_CORES):
        b, k = core // K, core % K
        contrib[b, k] = _unperm(
            np.asarray(res1.results[core]["yc"]).astype(np.float32), k)

    wgT = np.ascontiguousarray(
        (f8(Wout) * f8(ln_gamma)[None, :]).T.astype(np.float32))
    vgneg = (-(f8(Wout) @ f8(ln_gamma))).astype(np.float32)
    vbeta = (f8(Wout) @ f8(ln_beta)).astype(np.float32)
    dvs = Dv.sum(axis=0).astype(np.float32).reshape(DI, 1)
    onesM = np.full((DI, 1), 1.0 / DI, np.float32)
    ones1 = np.ones((1, DI), np.float32)
    yrow = [np.ascontiguousarray(y[b].reshape(DM, L)).astype(ml_dtypes.bfloat16)
            for b in range(B)]

    in_maps2 = []
    for core in range(N_CORES):
        b, q = core // K, core % K
        sl = slice(q * L2, (q + 1) * L2)
        m = {f"c{i}": np.ascontiguousarray(contrib[b, i][:, sl])
             for i in range(K)}
        m.update(ykq=np.ascontiguousarray(yrow[b][:, sl]),
                 wgT=wgT, wuT=np.ascontiguousarray(f8(Wy).T.astype(ml_dtypes.bfloat16)),
                 vg1=vgneg.reshape(1, DM), vb1=vbeta.reshape(1, DM),
                 dvs=dvs, onesM=onesM, ones1=ones1,
                 epsv=np.full((1, 1), LN_EPS, np.float32))
        in_maps2.append(m)

    res2 = bass_utils.run_bass_kernel_spmd(nc2, in_maps2,
                                           core_ids=list(range(N_CORES)))
    _CACHE["res2"] = res2

    out = np.empty((B, DM, L), np.float32)
    for core in range(N_CORES):
        b, q = core // K, core % K
        out[b][:, q * L2:(q + 1) * L2] = res2.results[core]["out2"]
    return out.reshape(B, DM, H, W)


# revision 24
# speedup vs baseline: 1.0031x; 1.0031x over previous
"""Trainium2 Bass kernel for nn_BranchFusSSM (VMamba-style cross-scan selective SSM).

Sharding: 8 cores = (batch b in {0,1}) x (scan direction k in {0..3}).

Launch 1 (per core): lane layout = 12 tiles of 128 lanes, lane (d, n) with
d = 8*t + p//16, n = p%16.  Per 1024-col block: input projections emit
delta-pre [96], u [96], and B/C directly in lane-replicated form [128]
(replication folded into the projection weights on host).  delta =
softplus (1 ACT op).  Per tile: delta-rep via PE selector matmul -> exp
(ACT, per-lane A scale); w = delta*u replicated via a 0-stride SBUF->SBUF
DMA (idle DMA engines); b = w*B and sty = h*C run as bf16 2x DVE ops;
tensor_tensor_scan split across DVE/Pool; n-contraction on PE into PSUM.
Scan-side elementwise ops process 2 blocks (2048 cols) per instruction.

Launch 2 (8 cores = (b, quarter-of-L)): sum 4 direction contributions
(bf16 2x adds), add (sum_k D_k) * u (u recomputed from y via PE),
LayerNorm over channels via PE ones-matmuls + row-stats, output
projection with gamma folded; the -mu*rstd*Wout@gamma + Wout@beta terms
enter the projection PSUM via a rank-2 outer-product matmul.

Host work between launches is pure permutation/slicing (numpy).
"""

import sys

if "/opt/trn_rl_repo" not in sys.path:
    sys.path.insert(0, "/opt/trn_rl_repo")

import numpy as np
import ml_dtypes

import concourse.bacc as bacc
import concourse.mybir as mybir
import concourse.hw_specs as _hw_specs
from concourse.tile import TileContext
from concourse import bass_utils

# Force every activation onto the one table containing exp+ln+copy+square
# (the only functions this kernel uses), so the scheduler never inserts ACT
# table loads (~2.7us per switch).
_ORIG_GAT = _hw_specs.get_activation_tables
_ONE_TABLE = "natural_log_exp_and_others"


def _gat_single(arch):
    full = _ORIG_GAT(arch)
    return {name: (funcs if name == _ONE_TABLE else set())
            for name, funcs in full.items()}


bacc.get_activation_tables = _gat_single

# problem constants (hardcoded per contract)
B = 2
DM = 96          # d_model
DI = 96          # d_inner
H = W = 128
L = H * W        # 16384
N = 16           # d_state
R = 6            # dt_rank
K = 4            # directions
LN_EPS = 1e-5

NT = 12          # lane tiles: 12 x (8 d-channels x 16 n) = 1536 states
DPT = 8          # d-channels per lane tile
BLK = 1024       # PSUM-side block
GRP = 2048       # SBUF-side group (2 blocks)
NGRP = L // GRP  # 8
F32 = mybir.dt.float32
F32R = mybir.dt.float32r
BF16 = mybir.dt.bfloat16

N_CORES = 8
ALU = mybir.AluOpType
AF = mybir.ActivationFunctionType

# tiles whose sb / sty muls run on Pool (rest on DVE as bf16-2x TT); the
# scan and TensorScalarPtr ops are DVE-only (illegal opcodes on Pool).
SB_POOL = 5
STY_POOL = 5


# ---------------------------------------------------------------------------
# host-side helpers
# ---------------------------------------------------------------------------

def _perm(t2d: np.ndarray, k: int) -> np.ndarray:
    """[C, H, W] image -> [C, L] sequence in direction-k scan order."""
    c = t2d.shape[0]
    if k == 0:
        return np.ascontiguousarray(t2d.reshape(c, L))
    if k == 1:
        return np.ascontiguousarray(t2d.transpose(0, 2, 1).reshape(c, L))
    if k == 2:
        return np.ascontiguousarray(t2d.reshape(c, L)[:, ::-1])
    return np.ascontiguousarray(t2d.transpose(0, 2, 1).reshape(c, L)[:, ::-1])


def _unperm(seq: np.ndarray, k: int) -> np.ndarray:
    """inverse of _perm: direction-k ordered [C, L] -> row-major [C, L]."""
    c = seq.shape[0]
    if k == 0:
        return seq
    if k == 1:
        return np.ascontiguousarray(seq.reshape(c, W, H).transpose(0, 2, 1).reshape(c, L))
    if k == 2:
        return np.ascontiguousarray(seq[:, ::-1])
    rev = seq[:, ::-1]
    return np.ascontiguousarray(rev.reshape(c, W, H).transpose(0, 2, 1).reshape(c, L))


def _lane_maps():
    """d(p) = DPT*t + p//N, n(p) = p%N for lane p of tile t."""
    p = np.arange(128)
    return p // N, p % N


def _selectors():
    pd, pn = _lane_maps()
    wsel = np.zeros((DI, NT * 128), np.float32)   # replicate d-rows into lanes
    ysel = np.zeros((128, NT * DI), np.float32)   # reduce lanes back to d-rows
    for t in range(NT):
        for p in range(128):
            d = DPT * t + pd[p]
            wsel[d, t * 128 + p] = 1.0
            ysel[p, t * DI + d] = 1.0
    return wsel, ysel


# ---------------------------------------------------------------------------
# launch 1: per-direction selective scan
# ---------------------------------------------------------------------------

def build_scan_program():
    nc = bacc.Bacc("TRN2", target_bir_lowering=False, debug=False)

    xk = nc.dram_tensor("xk", [DM, L], F32R, kind="ExternalInput")
    yk = nc.dram_tensor("yk", [DM, L], F32R, kind="ExternalInput")
    wdT = nc.dram_tensor("wdT", [DM, DI], F32R, kind="ExternalInput")
    wuT = nc.dram_tensor("wuT", [DM, DI], F32R, kind="ExternalInput")
    wBrepT = nc.dram_tensor("wBrepT", [DM, 128], F32R, kind="ExternalInput")
    wCrepT = nc.dram_tensor("wCrepT", [DM, 128], F32R, kind="ExternalInput")
    dtbias = nc.dram_tensor("dtbias", [DI, 1], F32, kind="ExternalInput")
    ascale = nc.dram_tensor("ascale", [128, NT], F32, kind="ExternalInput")
    wsel = nc.dram_tensor("wsel", [DI, NT * 128], BF16, kind="ExternalInput")
    ysel = nc.dram_tensor("ysel", [128, NT * DI], BF16, kind="ExternalInput")
    yc = nc.dram_tensor("yc", [DI, L], BF16, kind="ExternalOutput")

    with TileContext(nc) as tc:
        with (
            tc.tile_pool(name="smalls", bufs=1) as smalls,
            tc.tile_pool(name="io", bufs=3) as io,
            tc.tile_pool(name="stage", bufs=3) as stage,
            tc.tile_pool(name="lanes", bufs=3) as lanes,
            tc.tile_pool(name="psum", bufs=1, space="PSUM") as psum,
        ):
            s_wdT = smalls.tile([DM, DI], F32R, tag="wdT")
            s_wuT = smalls.tile([DM, DI], F32R, tag="wuT")
            s_wB = smalls.tile([DM, 128], F32R, tag="wB")
            s_wC = smalls.tile([DM, 128], F32R, tag="wC")
            s_bias = smalls.tile([DI, 1], F32, tag="bias")
            s_asc = smalls.tile([128, NT], F32, tag="asc")
            s_wsel = smalls.tile([DI, NT * 128], BF16, tag="wsel")
            s_ysel = smalls.tile([128, NT * DI], BF16, tag="ysel")
            nc.sync.dma_start(s_wdT[:], wdT.ap())
            nc.sync.dma_start(s_wuT[:], wuT.ap())
            nc.sync.dma_start(s_wB[:], wBrepT.ap())
            nc.sync.dma_start(s_wC[:], wCrepT.ap())
            nc.sync.dma_start(s_bias[:], dtbias.ap())
            nc.sync.dma_start(s_asc[:], ascale.ap())
            nc.sync.dma_start(s_wsel[:], wsel.ap())
            nc.sync.dma_start(s_ysel[:], ysel.ap())

            htails = [smalls.tile([128, 1], F32, tag=f"ht{t}", name=f"ht{t}")
                      for t in range(NT)]

            def h_loop(g):
                sdb2 = stage.tile([DI, GRP], BF16, tag="sdb2")
                sw2 = stage.tile([DI, GRP], BF16, tag="sw2")
                sB2 = stage.tile([128, GRP], BF16, tag="sB2")
                sC2 = stage.tile([128, GRP], BF16, tag="sC2")
                for h in range(2):
                    lo = g * GRP + h * BLK
                    hs = slice(h * BLK, (h + 1) * BLK)
                    xkb = io.tile([DM, BLK], F32R, tag="xkb")
                    ykb = io.tile([DM, BLK], F32R, tag="ykb")
                    nc.sync.dma_start(xkb[:], xk.ap()[:, lo:lo + BLK])
                    nc.sync.dma_start(ykb[:], yk.ap()[:, lo:lo + BLK])

                    pd_ = psum.tile([128, BLK], F32, tag="ps", bufs=3)
                    for c0 in range(0, BLK, 512):
                        nc.tensor.matmul(pd_[0:DI, c0:c0 + 512], s_wdT[:],
                                         xkb[:, c0:c0 + 512],
                                         start=True, stop=True)
                    # delta = softplus(z) = ln(exp(z + bias) + 1)
                    sez = io.tile([DI, BLK], BF16, tag="sez")
                    nc.scalar.activation(sez[:], pd_[0:DI, :], AF.Exp,
                                         bias=s_bias[:], scale=1.0)
                    nc.scalar.activation(sdb2[:, hs], sez[:], AF.Ln, bias=1.0)

                    pu = psum.tile([128, BLK], F32, tag="ps", bufs=3)
                    for c0 in range(0, BLK, 512):
                        nc.tensor.matmul(pu[0:DI, c0:c0 + 512], s_wuT[:],
                                         ykb[:, c0:c0 + 512],
                                         start=True, stop=True)
                    nc.vector.tensor_mul(sw2[:, hs], sdb2[:, hs], pu[0:DI, :])

                    pB = psum.tile([128, BLK], F32, tag="ps", bufs=3)
                    for c0 in range(0, BLK, 512):
                        nc.tensor.matmul(pB[:, c0:c0 + 512], s_wB[:],
                                         xkb[:, c0:c0 + 512],
                                         start=True, stop=True)
                    nc.scalar.copy(sB2[:, hs], pB[:])

                    pC = psum.tile([128, BLK], F32, tag="ps", bufs=3)
                    for c0 in range(0, BLK, 512):
                        nc.tensor.matmul(pC[:, c0:c0 + 512], s_wC[:],
                                         xkb[:, c0:c0 + 512],
                                         start=True, stop=True)
                    nc.scalar.copy(sC2[:, hs], pC[:])
                return sdb2, sw2, sB2, sC2

            def tile_loop(g, sdb2, sw2, sB2, sC2):
                stys = []
                for t in range(NT):
                    # w replicated into lanes via 0-stride SBUF->SBUF DMA
                    swrep = lanes.tile([128, GRP], BF16, tag="swrep", bufs=3)
                    nc.sync.dma_start(
                        swrep[:],
                        sw2[DPT * t:DPT * t + DPT, :].unsqueeze(1)
                            .broadcast_to([DPT, N, GRP]))

                    sa2 = lanes.tile([128, GRP], F32, tag="sa", bufs=3)
                    for h in range(2):
                        hs = slice(h * BLK, (h + 1) * BLK)
                        pa = psum.tile([128, BLK], F32, tag="ps", bufs=3)
                        for c0 in range(0, BLK, 512):
                            nc.tensor.matmul(
                                pa[:, c0:c0 + 512],
                                s_wsel[:, t * 128:(t + 1) * 128],
                                sdb2[:, h * BLK + c0:h * BLK + c0 + 512],
                                start=True, stop=True)
                        nc.scalar.activation(sa2[:, hs], pa[:], AF.Exp,
                                             scale=s_asc[:, t:t + 1])

                    sb2 = lanes.tile([128, GRP], BF16, tag="sb", bufs=3)
                    if t >= NT - SB_POOL:
                        nc.gpsimd.tensor_mul(sb2[:, 0:BLK], swrep[:, 0:BLK],
                                             sB2[:, 0:BLK])
                        nc.gpsimd.tensor_mul(sb2[:, BLK:GRP], swrep[:, BLK:GRP],
                                             sB2[:, BLK:GRP])
                    else:
                        nc.vector.tensor_mul(sb2[:], swrep[:], sB2[:])

                    sh2 = lanes.tile([128, GRP], BF16, tag="sh", bufs=3)
                    init = 0.0 if g == 0 else htails[t][:]
                    nc.vector.tensor_tensor_scan(sh2[:], sa2[:], sb2[:], init,
                                                 op0=ALU.mult, op1=ALU.add)
                    nc.vector.tensor_scalar_add(htails[t][:],
                                                sh2[:, GRP - 1:GRP], 0.0)

                    sty2 = lanes.tile([128, GRP], BF16, tag=f"sty{t}",
                                      bufs=1, name=f"sty{t}")
                    if t < STY_POOL:
                        nc.gpsimd.tensor_mul(sty2[:, 0:BLK], sh2[:, 0:BLK],
                                             sC2[:, 0:BLK])
                        nc.gpsimd.tensor_mul(sty2[:, BLK:GRP], sh2[:, BLK:GRP],
                                             sC2[:, BLK:GRP])
                    else:
                        nc.vector.tensor_mul(sty2[:], sh2[:], sC2[:])
                    stys.append(sty2)
                return stys

            def y_batch(g, stys):
                # y-contraction issued after the NEXT group's projections so
                # its operand waits never stall PE's in-order queue
                for h in range(2):
                    lo = g * GRP + h * BLK
                    yp = psum.tile([DI, BLK], F32, tag="yacc", bufs=1,
                                   name=f"yp{g}_{h}")
                    for t in range(NT):
                        for c0 in range(0, BLK, 512):
                            nc.tensor.matmul(
                                yp[:, c0:c0 + 512],
                                s_ysel[:, t * DI:(t + 1) * DI],
                                stys[t][:, h * BLK + c0:h * BLK + c0 + 512],
                                start=(t == 0), stop=(t == NT - 1))
                    syc = io.tile([DI, BLK], BF16, tag="syc")
                    nc.scalar.copy(syc[:], yp[:])
                    nc.scalar.dma_start(yc.ap()[:, lo:lo + BLK], syc[:])

            stage_t = h_loop(0)
            for g in range(NGRP):
                stys = tile_loop(g, *stage_t)
                if g + 1 < NGRP:
                    stage_t = h_loop(g + 1)
                y_batch(g, stys)

    nc.compile()
    return nc


# ---------------------------------------------------------------------------
# launch 2: merge 4 directions + D*u + LayerNorm + output projection
# ---------------------------------------------------------------------------

L2 = L // 4      # positions per core: 4096
C2 = 1024        # processing chunk


def build_merge_program():
    nc = bacc.Bacc("TRN2", target_bir_lowering=False, debug=False)

    cin = [nc.dram_tensor(f"c{i}", [DI, L2], F32R, kind="ExternalInput")
           for i in range(K)]
    ykq = nc.dram_tensor("ykq", [DM, L2], BF16, kind="ExternalInput")
    wgT = nc.dram_tensor("wgT", [DI, DM], F32R, kind="ExternalInput")
    wuT = nc.dram_tensor("wuT", [DM, DI], BF16, kind="ExternalInput")
    vg1 = nc.dram_tensor("vg1", [1, DM], F32R, kind="ExternalInput")
    vb1 = nc.dram_tensor("vb1", [1, DM], F32R, kind="ExternalInput")
    dvs = nc.dram_tensor("dvs", [DI, 1], F32, kind="ExternalInput")
    onesM = nc.dram_tensor("onesM", [DI, 1], F32R, kind="ExternalInput")
    ones1 = nc.dram_tensor("ones1", [1, DI], F32R, kind="ExternalInput")
    epsv = nc.dram_tensor("epsv", [1, 1], F32, kind="ExternalInput")
    out2 = nc.dram_tensor("out2", [DM, L2], F32, kind="ExternalOutput")

    with TileContext(nc) as tc:
        with (
            tc.tile_pool(name="smalls", bufs=1) as smalls,
            tc.tile_pool(name="work", bufs=3) as work,
            tc.tile_pool(name="psum", bufs=1, space="PSUM") as psum,
        ):
            s_wgT = smalls.tile([DI, DM], F32R, tag="wgT")
            s_wuT = smalls.tile([DM, DI], BF16, tag="wuT")
            s_vg1 = smalls.tile([1, DM], F32R, tag="vg1")
            s_vb1 = smalls.tile([1, DM], F32R, tag="vb1")
            s_dvs = smalls.tile([DI, 1], F32, tag="dvs")
            s_ones = smalls.tile([DI, 1], F32R, tag="ones")
            s_ones1 = smalls.tile([1, DI], F32R, tag="ones1")
            s_eps = smalls.tile([1, 1], F32, tag="eps")
            nc.sync.dma_start(s_wgT[:], wgT.ap())
            nc.sync.dma_start(s_wuT[:], wuT.ap())
            nc.sync.dma_start(s_vg1[:], vg1.ap())
            nc.sync.dma_start(s_vb1[:], vb1.ap())
            nc.sync.dma_start(s_dvs[:], dvs.ap())
            nc.sync.dma_start(s_ones[:], onesM.ap())
            nc.sync.dma_start(s_ones1[:], ones1.ap())
            nc.sync.dma_start(s_eps[:], epsv.ap())

            for j in range(L2 // C2):
                sl = slice(j * C2, (j + 1) * C2)
                cb = []
                for i in range(K):
                    t = work.tile([DI, C2], F32R, tag=f"cin{i}", name=f"cin{i}")
                    nc.sync.dma_start(t[:], cin[i].ap()[:, sl])
                    cb.append(t)
                ykb = work.tile([DM, C2], BF16, tag="ykb")
                nc.sync.dma_start(ykb[:], ykq.ap()[:, sl])

                t01 = work.tile([DI, C2], F32, tag="t01")
                t23 = work.tile([DI, C2], F32, tag="t23")
                s4b = work.tile([DI, C2], F32, tag="s4b")
                nc.vector.tensor_add(t01[:], cb[0][:], cb[1][:])
                nc.vector.tensor_add(t23[:], cb[2][:], cb[3][:])
                nc.vector.tensor_add(s4b[:], t01[:], t23[:])

                # u = Wy @ y (row-major), s4 = s4b + (sum_k D_k) * u
                pu = psum.tile([DI, C2], F32, tag="m96", bufs=1)
                for c0 in range(0, C2, 512):
                    nc.tensor.matmul(pu[:, c0:c0 + 512], s_wuT[:],
                                     ykb[:, c0:c0 + 512],
                                     start=True, stop=True)
                s4 = work.tile([DI, C2], F32R, tag="s4")
                nc.vector.scalar_tensor_tensor(s4[:], pu[:], s_dvs[:], s4b[:],
                                               op0=ALU.mult, op1=ALU.add)

                ssq = work.tile([DI, C2], F32R, tag="ssq")
                nc.scalar.activation(ssq[:], s4[:], AF.Square)

                pmu = psum.tile([1, C2], F32, tag="pmu", bufs=1)
                psq = psum.tile([1, C2], F32, tag="psq", bufs=1)
                for c0 in range(0, C2, 512):
                    nc.tensor.matmul(pmu[:, c0:c0 + 512], s_ones[:],
                                     s4[:, c0:c0 + 512],
                                     start=True, stop=True)
                    nc.tensor.matmul(psq[:, c0:c0 + 512], s_ones[:],
                                     ssq[:, c0:c0 + 512],
                                     start=True, stop=True)
                smusq = work.tile([1, C2], F32, tag="smusq")
                nc.scalar.activation(smusq[:], pmu[:], AF.Square)
                svar = work.tile([1, C2], F32, tag="svar")
                nc.vector.tensor_sub(svar[:], psq[:], smusq[:])
                # rsqrt(var+eps) = exp(-0.5 * ln(var+eps))
                slnv = work.tile([1, C2], F32, tag="slnv")
                nc.scalar.activation(slnv[:], svar[:], AF.Ln, bias=s_eps[:])
                srstd = work.tile([1, C2], F32R, tag="srstd")
                nc.scalar.activation(srstd[:], slnv[:], AF.Exp, scale=-0.5)
                srcp = work.tile([1, C2], F32R, tag="srcp")
                nc.scalar.activation(srcp[:], slnv[:], AF.Exp, scale=0.5)
                smu = work.tile([1, C2], F32R, tag="smu")
                nc.scalar.copy(smu[:], pmu[:])

                # B1 = broadcast rstd over 96 channel rows
                pB1 = psum.tile([DI, C2], F32, tag="m96", bufs=1)
                for c0 in range(0, C2, 512):
                    nc.tensor.matmul(pB1[:, c0:c0 + 512], s_ones1[:],
                                     srstd[:, c0:c0 + 512],
                                     start=True, stop=True)
                sB1 = work.tile([DM, C2], BF16, tag="sB1")
                nc.scalar.copy(sB1[:], pB1[:])

                # pq = Wout·gamma @ s4 + vgneg (x) mu + vbeta (x) (1/rstd);
                # the final multiply by the broadcast rstd then yields
                # rstd*pq + vgneg*(mu*rstd) + vbeta.
                pq = psum.tile([DM, C2], F32, tag="pq", bufs=1)
                for c0 in range(0, C2, 512):
                    nc.tensor.matmul(pq[:, c0:c0 + 512], s_wgT[:],
                                     s4[:, c0:c0 + 512],
                                     start=True, stop=False)
                    nc.tensor.matmul(pq[:, c0:c0 + 512], s_vg1[:],
                                     smu[:, c0:c0 + 512],
                                     start=False, stop=False)
                    nc.tensor.matmul(pq[:, c0:c0 + 512], s_vb1[:],
                                     srcp[:, c0:c0 + 512],
                                     start=False, stop=True)
                so = work.tile([DM, C2], F32, tag="so")
                nc.vector.tensor_mul(so[:], pq[:], sB1[:])
                nc.sync.dma_start(out2.ap()[:, sl], so[:])

    nc.compile()
    return nc


# ---------------------------------------------------------------------------
# host orchestration
# ---------------------------------------------------------------------------

_CACHE: dict = {}


def _programs():
    if "p1" not in _CACHE:
        _CACHE["p1"] = build_scan_program()
        _CACHE["p2"] = build_merge_program()
    return _CACHE["p1"], _CACHE["p2"]


def kernel(x, y, Wx, Wy, x_proj_weight, dt_projs_weight, dt_projs_bias,
           A_logs, Ds, ln_gamma, ln_beta, Wout):
    x = np.asarray(x, np.float32)
    y = np.asarray(y, np.float32)
    f8 = lambda a: np.asarray(a, np.float64)

    wsel_np, ysel_np = _selectors()
    pd, pn = _lane_maps()
    A = -np.exp(f8(A_logs)).reshape(K, DI, N)
    Dv = f8(Ds).reshape(K, DI)

    nc1, nc2 = _programs()

    in_maps1 = []
    for core in range(N_CORES):
        b, k = core // K, core % K
        Wd = (f8(dt_projs_weight)[k] @ f8(x_proj_weight)[k][:R] @ f8(Wx))
        WB = f8(x_proj_weight)[k][R:R + N] @ f8(Wx)     # [N, DM]
        WC = f8(x_proj_weight)[k][R + N:] @ f8(Wx)

        asc = np.empty((128, NT), np.float32)
        for t in range(NT):
            asc[:, t] = A[k][DPT * t + pd, pn]

        in_maps1.append(dict(
            xk=_perm(x[b], k),
            yk=_perm(y[b], k),
            wdT=np.ascontiguousarray(Wd.T.astype(np.float32)),
            wuT=np.ascontiguousarray(f8(Wy).T.astype(np.float32)),
            wBrepT=np.ascontiguousarray(WB[pn].T.astype(np.float32)),
            wCrepT=np.ascontiguousarray(WC[pn].T.astype(np.float32)),
            dtbias=np.asarray(dt_projs_bias, np.float32)[k].reshape(DI, 1),
            ascale=asc,
            wsel=wsel_np.astype(ml_dtypes.bfloat16),
            ysel=ysel_np.astype(ml_dtypes.bfloat16),
        ))

    res1 = bass_utils.run_bass_kernel_spmd(nc1, in_maps1,
                                           core_ids=list(range(N_CORES)))
    _CACHE["res1"] = res1

    # un-permute each direction's contribution back to row-major order
    contrib = np.empty((B, K, DI, L), np.float32)
    for core in range(N_CORES):
        b, k = core // K, core % K
        contrib[b, k] = _unperm(
            np.asarray(res1.results[core]["yc"]).astype(np.float32), k)

    wgT = np.ascontiguousarray(
        (f8(Wout) * f8(ln_gamma)[None, :]).T.astype(np.float32))
    vgneg = (-(f8(Wout) @ f8(ln_gamma))).astype(np.float32)
    vbeta = (f8(Wout) @ f8(ln_beta)).astype(np.float32)
    dvs = Dv.sum(axis=0).astype(np.float32).reshape(DI, 1)
    onesM = np.full((DI, 1), 1.0 / DI, np.float32)
    ones1 = np.ones((1, DI), np.float32)
    yrow = [np.ascontiguousarray(y[b].reshape(DM, L)).astype(ml_dtypes.bfloat16)
            for b in range(B)]

    in_maps2 = []
    for core in range(N_CORES):
        b, q = core // K, core % K
        sl = slice(q * L2, (q + 1) * L2)
        m = {f"c{i}": np.ascontiguousarray(contrib[b, i][:, sl])
             for i in range(K)}
        m.update(ykq=np.ascontiguousarray(yrow[b][:, sl]),
                 wgT=wgT, wuT=np.ascontiguousarray(f8(Wy).T.astype(ml_dtypes.bfloat16)),
                 vg1=vgneg.reshape(1, DM), vb1=vbeta.reshape(1, DM),
                 dvs=dvs, onesM=onesM, ones1=ones1,
                 epsv=np.full((1, 1), LN_EPS, np.float32))
        in_maps2.append(m)

    res2 = bass_utils.run_bass_kernel_spmd(nc2, in_maps2,
                                           core_ids=list(range(N_CORES)))
    _CACHE["res2"] = res2

    out = np.empty((B, DM, L), np.float32)
    for core in range(N_CORES):
        b, q = core // K, core % K
        out[b][:, q * L2:(q + 1) * L2] = res2.results[core]["out2"]
    return out.reshape(B, DM, H, W)


# revision 25
# speedup vs baseline: 1.0144x; 1.0112x over previous
"""Trainium2 Bass kernel for nn_BranchFusSSM (VMamba-style cross-scan selective SSM).

Sharding: 8 cores = (batch b in {0,1}) x (scan direction k in {0..3}).

Launch 1 (per core): lane layout = 12 tiles of 128 lanes, lane (d, n) with
d = 8*t + p//16, n = p%16.  Per 1024-col block: input projections emit
delta-pre [96], u [96], and B/C directly in lane-replicated form [128]
(replication folded into the projection weights on host).  delta =
softplus (1 ACT op).  Per tile: delta-rep via PE selector matmul -> exp
(ACT, per-lane A scale); w = delta*u replicated via a 0-stride SBUF->SBUF
DMA (idle DMA engines); b = w*B and sty = h*C run as bf16 2x DVE ops;
tensor_tensor_scan split across DVE/Pool; n-contraction on PE into PSUM.
Scan-side elementwise ops process 2 blocks (2048 cols) per instruction.

Launch 2 (8 cores = (b, quarter-of-L)): sum 4 direction contributions
(bf16 2x adds), add (sum_k D_k) * u (u recomputed from y via PE),
LayerNorm over channels via PE ones-matmuls + row-stats, output
projection with gamma folded; the -mu*rstd*Wout@gamma + Wout@beta terms
enter the projection PSUM via a rank-2 outer-product matmul.

Host work between launches is pure permutation/slicing (numpy).
"""

import sys

if "/opt/trn_rl_repo" not in sys.path:
    sys.path.insert(0, "/opt/trn_rl_repo")

import numpy as np
import ml_dtypes

import concourse.bacc as bacc
import concourse.mybir as mybir
import concourse.hw_specs as _hw_specs
from concourse.tile import TileContext
from concourse import bass_utils

# Force every activation onto the one table containing exp+ln+copy+square
# (the only functions this kernel uses), so the scheduler never inserts ACT
# table loads (~2.7us per switch).
_ORIG_GAT = _hw_specs.get_activation_tables
_ONE_TABLE = "natural_log_exp_and_others"


def _gat_single(arch):
    full = _ORIG_GAT(arch)
    return {name: (funcs if name == _ONE_TABLE else set())
            for name, funcs in full.items()}


bacc.get_activation_tables = _gat_single

# problem constants (hardcoded per contract)
B = 2
DM = 96          # d_model
DI = 96          # d_inner
H = W = 128
L = H * W        # 16384
N = 16           # d_state
R = 6            # dt_rank
K = 4            # directions
LN_EPS = 1e-5

NT = 12          # lane tiles: 12 x (8 d-channels x 16 n) = 1536 states
DPT = 8          # d-channels per lane tile
BLK = 1024       # PSUM-side block
GRP = 2048       # SBUF-side group (2 blocks)
NGRP = L // GRP  # 8
F32 = mybir.dt.float32
F32R = mybir.dt.float32r
BF16 = mybir.dt.bfloat16

N_CORES = 8
ALU = mybir.AluOpType
AF = mybir.ActivationFunctionType

# tiles whose sb / sty muls run on Pool (rest on DVE as bf16-2x TT); the
# scan and TensorScalarPtr ops are DVE-only (illegal opcodes on Pool).
SB_POOL = 5
STY_POOL = 5


# ---------------------------------------------------------------------------
# host-side helpers
# ---------------------------------------------------------------------------

def _perm(t2d: np.ndarray, k: int) -> np.ndarray:
    """[C, H, W] image -> [C, L] sequence in direction-k scan order."""
    c = t2d.shape[0]
    if k == 0:
        return np.ascontiguousarray(t2d.reshape(c, L))
    if k == 1:
        return np.ascontiguousarray(t2d.transpose(0, 2, 1).reshape(c, L))
    if k == 2:
        return np.ascontiguousarray(t2d.reshape(c, L)[:, ::-1])
    return np.ascontiguousarray(t2d.transpose(0, 2, 1).reshape(c, L)[:, ::-1])


def _unperm(seq: np.ndarray, k: int) -> np.ndarray:
    """inverse of _perm: direction-k ordered [C, L] -> row-major [C, L]."""
    c = seq.shape[0]
    if k == 0:
        return seq
    if k == 1:
        return np.ascontiguousarray(seq.reshape(c, W, H).transpose(0, 2, 1).reshape(c, L))
    if k == 2:
        return np.ascontiguousarray(seq[:, ::-1])
    rev = seq[:, ::-1]
    return np.ascontiguousarray(rev.reshape(c, W, H).transpose(0, 2, 1).reshape(c, L))


def _lane_maps():
    """d(p) = DPT*t + p//N, n(p) = p%N for lane p of tile t."""
    p = np.arange(128)
    return p // N, p % N


def _selectors():
    pd, pn = _lane_maps()
    wsel = np.zeros((DI, NT * 128), np.float32)   # replicate d-rows into lanes
    ysel = np.zeros((128, NT * DI), np.float32)   # reduce lanes back to d-rows
    for t in range(NT):
        for p in range(128):
            d = DPT * t + pd[p]
            wsel[d, t * 128 + p] = 1.0
            ysel[p, t * DI + d] = 1.0
    return wsel, ysel


# ---------------------------------------------------------------------------
# launch 1: per-direction selective scan
# ---------------------------------------------------------------------------

def build_scan_program():
    nc = bacc.Bacc("TRN2", target_bir_lowering=False, debug=False)

    xk = nc.dram_tensor("xk", [DM, L], F32R, kind="ExternalInput")
    yk = nc.dram_tensor("yk", [DM, L], F32R, kind="ExternalInput")
    wdT = nc.dram_tensor("wdT", [DM, DI], F32R, kind="ExternalInput")
    wuT = nc.dram_tensor("wuT", [DM, DI], F32R, kind="ExternalInput")
    wBrepT = nc.dram_tensor("wBrepT", [DM, 128], F32R, kind="ExternalInput")
    wCrepT = nc.dram_tensor("wCrepT", [DM, 128], F32R, kind="ExternalInput")
    dtbias = nc.dram_tensor("dtbias", [DI, 1], F32, kind="ExternalInput")
    ascale = nc.dram_tensor("ascale", [128, NT], F32, kind="ExternalInput")
    wsel = nc.dram_tensor("wsel", [DI, NT * 128], BF16, kind="ExternalInput")
    ysel = nc.dram_tensor("ysel", [128, NT * DI], BF16, kind="ExternalInput")
    yc = nc.dram_tensor("yc", [DI, L], BF16, kind="ExternalOutput")

    with TileContext(nc) as tc:
        with (
            tc.tile_pool(name="smalls", bufs=1) as smalls,
            tc.tile_pool(name="io", bufs=3) as io,
            tc.tile_pool(name="stage", bufs=3) as stage,
            tc.tile_pool(name="lanes", bufs=3) as lanes,
            tc.tile_pool(name="psum", bufs=1, space="PSUM") as psum,
        ):
            s_wdT = smalls.tile([DM, DI], F32R, tag="wdT")
            s_wuT = smalls.tile([DM, DI], F32R, tag="wuT")
            s_wB = smalls.tile([DM, 128], F32R, tag="wB")
            s_wC = smalls.tile([DM, 128], F32R, tag="wC")
            s_bias = smalls.tile([DI, 1], F32, tag="bias")
            s_asc = smalls.tile([128, NT], F32, tag="asc")
            s_wsel = smalls.tile([DI, NT * 128], BF16, tag="wsel")
            s_ysel = smalls.tile([128, NT * DI], BF16, tag="ysel")
            nc.sync.dma_start(s_wdT[:], wdT.ap())
            nc.sync.dma_start(s_wuT[:], wuT.ap())
            nc.sync.dma_start(s_wB[:], wBrepT.ap())
            nc.sync.dma_start(s_wC[:], wCrepT.ap())
            nc.sync.dma_start(s_bias[:], dtbias.ap())
            nc.sync.dma_start(s_asc[:], ascale.ap())
            nc.sync.dma_start(s_wsel[:], wsel.ap())
            nc.sync.dma_start(s_ysel[:], ysel.ap())

            htails = [smalls.tile([128, 1], F32, tag=f"ht{t}", name=f"ht{t}")
                      for t in range(NT)]

            def h_loop(g):
                sdb2 = stage.tile([DI, GRP], BF16, tag="sdb2")
                sw2 = stage.tile([DI, GRP], BF16, tag="sw2")
                sB2 = stage.tile([128, GRP], BF16, tag="sB2")
                sC2 = stage.tile([128, GRP], BF16, tag="sC2")
                for h in range(2):
                    lo = g * GRP + h * BLK
                    hs = slice(h * BLK, (h + 1) * BLK)
                    xkb = io.tile([DM, BLK], F32R, tag="xkb")
                    ykb = io.tile([DM, BLK], F32R, tag="ykb")
                    nc.sync.dma_start(xkb[:], xk.ap()[:, lo:lo + BLK])
                    nc.sync.dma_start(ykb[:], yk.ap()[:, lo:lo + BLK])

                    pd_ = psum.tile([128, BLK], F32, tag="ps", bufs=3)
                    for c0 in range(0, BLK, 512):
                        nc.tensor.matmul(pd_[0:DI, c0:c0 + 512], s_wdT[:],
                                         xkb[:, c0:c0 + 512],
                                         start=True, stop=True)
                    # delta = softplus(z) = ln(exp(z + bias) + 1)
                    sez = io.tile([DI, BLK], BF16, tag="sez")
                    nc.scalar.activation(sez[:], pd_[0:DI, :], AF.Exp,
                                         bias=s_bias[:], scale=1.0)
                    nc.scalar.activation(sdb2[:, hs], sez[:], AF.Ln, bias=1.0)

                    pu = psum.tile([128, BLK], F32, tag="ps", bufs=3)
                    for c0 in range(0, BLK, 512):
                        nc.tensor.matmul(pu[0:DI, c0:c0 + 512], s_wuT[:],
                                         ykb[:, c0:c0 + 512],
                                         start=True, stop=True)
                    nc.vector.tensor_mul(sw2[:, hs], sdb2[:, hs], pu[0:DI, :])

                    pB = psum.tile([128, BLK], F32, tag="ps", bufs=3)
                    for c0 in range(0, BLK, 512):
                        nc.tensor.matmul(pB[:, c0:c0 + 512], s_wB[:],
                                         xkb[:, c0:c0 + 512],
                                         start=True, stop=True)
                    nc.scalar.copy(sB2[:, hs], pB[:])

                    pC = psum.tile([128, BLK], F32, tag="ps", bufs=3)
                    for c0 in range(0, BLK, 512):
                        nc.tensor.matmul(pC[:, c0:c0 + 512], s_wC[:],
                                         xkb[:, c0:c0 + 512],
                                         start=True, stop=True)
                    nc.scalar.copy(sC2[:, hs], pC[:])
                return sdb2, sw2, sB2, sC2

            def tile_loop(g, sdb2, sw2, sB2, sC2):
                stys = []
                for t in range(NT):
                    # w replicated into lanes via 0-stride SBUF->SBUF DMA
                    swrep = lanes.tile([128, GRP], BF16, tag="swrep", bufs=3)
                    nc.sync.dma_start(
                        swrep[:],
                        sw2[DPT * t:DPT * t + DPT, :].unsqueeze(1)
                            .broadcast_to([DPT, N, GRP]))

                    sa2 = lanes.tile([128, GRP], F32, tag="sa", bufs=3)
                    for h in range(2):
                        hs = slice(h * BLK, (h + 1) * BLK)
                        pa = psum.tile([128, BLK], F32, tag="ps", bufs=3)
                        for c0 in range(0, BLK, 512):
                            nc.tensor.matmul(
                                pa[:, c0:c0 + 512],
                                s_wsel[:, t * 128:(t + 1) * 128],
                                sdb2[:, h * BLK + c0:h * BLK + c0 + 512],
                                start=True, stop=True)
                        nc.scalar.activation(sa2[:, hs], pa[:], AF.Exp,
                                             scale=s_asc[:, t:t + 1])

                    sb2 = lanes.tile([128, GRP], BF16, tag="sb", bufs=3)
                    if t >= NT - SB_POOL:
                        nc.gpsimd.tensor_mul(sb2[:, 0:BLK], swrep[:, 0:BLK],
                                             sB2[:, 0:BLK])
                        nc.gpsimd.tensor_mul(sb2[:, BLK:GRP], swrep[:, BLK:GRP],
                                             sB2[:, BLK:GRP])
                    else:
                        nc.vector.tensor_mul(sb2[:], swrep[:], sB2[:])

                    sh2 = lanes.tile([128, GRP], BF16, tag="sh", bufs=3)
                    init = 0.0 if g == 0 else htails[t][:]
                    nc.vector.tensor_tensor_scan(sh2[:], sa2[:], sb2[:], init,
                                                 op0=ALU.mult, op1=ALU.add)
                    nc.vector.tensor_scalar_add(htails[t][:],
                                                sh2[:, GRP - 1:GRP], 0.0)

                    sty2 = lanes.tile([128, GRP], BF16, tag=f"sty{t}",
                                      bufs=1, name=f"sty{t}")
                    if t < STY_POOL:
                        nc.gpsimd.tensor_mul(sty2[:, 0:BLK], sh2[:, 0:BLK],
                                             sC2[:, 0:BLK])
                        nc.gpsimd.tensor_mul(sty2[:, BLK:GRP], sh2[:, BLK:GRP],
                                             sC2[:, BLK:GRP])
                    else:
                        nc.vector.tensor_mul(sty2[:], sh2[:], sC2[:])
                    stys.append(sty2)
                return stys

            def y_batch(g, stys):
                # y-contraction issued after the NEXT group's projections so
                # its operand waits never stall PE's in-order queue
                for h in range(2):
                    lo = g * GRP + h * BLK
                    yp = psum.tile([DI, BLK], F32, tag="yacc", bufs=1,
                                   name=f"yp{g}_{h}")
                    for t in range(NT):
                        for c0 in range(0, BLK, 512):
                            nc.tensor.matmul(
                                yp[:, c0:c0 + 512],
                                s_ysel[:, t * DI:(t + 1) * DI],
                                stys[t][:, h * BLK + c0:h * BLK + c0 + 512],
                                start=(t == 0), stop=(t == NT - 1))
                    syc = io.tile([DI, BLK], BF16, tag="syc")
                    nc.scalar.copy(syc[:], yp[:])
                    nc.scalar.dma_start(yc.ap()[:, lo:lo + BLK], syc[:])

            stage_t = h_loop(0)
            for g in range(NGRP):
                stys = tile_loop(g, *stage_t)
                if g + 1 < NGRP:
                    stage_t = h_loop(g + 1)
                y_batch(g, stys)

    nc.compile()
    return nc


# ---------------------------------------------------------------------------
# launch 2: merge 4 directions + D*u + LayerNorm + output projection
# ---------------------------------------------------------------------------

L2 = L // 4      # positions per core: 4096
C2 = 512         # processing chunk (two alternating PSUM tag families)


def build_merge_program():
    nc = bacc.Bacc("TRN2", target_bir_lowering=False, debug=False)

    cin = [nc.dram_tensor(f"c{i}", [DI, L2], F32R, kind="ExternalInput")
           for i in range(K)]
    ykq = nc.dram_tensor("ykq", [DM, L2], BF16, kind="ExternalInput")
    wgT = nc.dram_tensor("wgT", [DI, DM], F32R, kind="ExternalInput")
    wuT = nc.dram_tensor("wuT", [DM, DI], BF16, kind="ExternalInput")
    vg1 = nc.dram_tensor("vg1", [1, DM], F32R, kind="ExternalInput")
    vb1 = nc.dram_tensor("vb1", [1, DM], F32R, kind="ExternalInput")
    dvs = nc.dram_tensor("dvs", [DI, 1], F32, kind="ExternalInput")
    onesM = nc.dram_tensor("onesM", [DI, 1], F32R, kind="ExternalInput")
    ones1 = nc.dram_tensor("ones1", [1, DI], F32R, kind="ExternalInput")
    epsv = nc.dram_tensor("epsv", [1, 1], F32, kind="ExternalInput")
    out2 = nc.dram_tensor("out2", [DM, L2], F32, kind="ExternalOutput")

    with TileContext(nc) as tc:
        with (
            tc.tile_pool(name="smalls", bufs=1) as smalls,
            tc.tile_pool(name="work", bufs=3) as work,
            tc.tile_pool(name="psum", bufs=1, space="PSUM") as psum,
        ):
            s_wgT = smalls.tile([DI, DM], F32R, tag="wgT")
            s_wuT = smalls.tile([DM, DI], BF16, tag="wuT")
            s_vg1 = smalls.tile([1, DM], F32R, tag="vg1")
            s_vb1 = smalls.tile([1, DM], F32R, tag="vb1")
            s_dvs = smalls.tile([DI, 1], F32, tag="dvs")
            s_ones = smalls.tile([DI, 1], F32R, tag="ones")
            s_ones1 = smalls.tile([1, DI], F32R, tag="ones1")
            s_eps = smalls.tile([1, 1], F32, tag="eps")
            nc.sync.dma_start(s_wgT[:], wgT.ap())
            nc.sync.dma_start(s_wuT[:], wuT.ap())
            nc.sync.dma_start(s_vg1[:], vg1.ap())
            nc.sync.dma_start(s_vb1[:], vb1.ap())
            nc.sync.dma_start(s_dvs[:], dvs.ap())
            nc.sync.dma_start(s_ones[:], onesM.ap())
            nc.sync.dma_start(s_ones1[:], ones1.ap())
            nc.sync.dma_start(s_eps[:], epsv.ap())

            for j in range(L2 // C2):
                sl = slice(j * C2, (j + 1) * C2)
                cb = []
                for i in range(K):
                    t = work.tile([DI, C2], F32R, tag=f"cin{i}", name=f"cin{i}")
                    nc.sync.dma_start(t[:], cin[i].ap()[:, sl])
                    cb.append(t)
                ykb = work.tile([DM, C2], BF16, tag="ykb")
                nc.sync.dma_start(ykb[:], ykq.ap()[:, sl])

                t01 = work.tile([DI, C2], F32, tag="t01")
                t23 = work.tile([DI, C2], F32, tag="t23")
                s4b = work.tile([DI, C2], F32, tag="s4b")
                nc.vector.tensor_add(t01[:], cb[0][:], cb[1][:])
                nc.vector.tensor_add(t23[:], cb[2][:], cb[3][:])
                nc.vector.tensor_add(s4b[:], t01[:], t23[:])

                # u = Wy @ y (row-major), s4 = s4b + (sum_k D_k) * u
                fam = j % 2
                pu = psum.tile([DI, C2], F32, tag=f"m96_{fam}", bufs=1)
                nc.tensor.matmul(pu[:], s_wuT[:], ykb[:],
                                 start=True, stop=True)
                s4 = work.tile([DI, C2], F32R, tag="s4")
                nc.vector.scalar_tensor_tensor(s4[:], pu[:], s_dvs[:], s4b[:],
                                               op0=ALU.mult, op1=ALU.add)

                ssq = work.tile([DI, C2], F32R, tag="ssq")
                nc.scalar.activation(ssq[:], s4[:], AF.Square)

                pmu = psum.tile([1, C2], F32, tag=f"pmu_{fam}", bufs=1)
                psq = psum.tile([1, C2], F32, tag=f"psq_{fam}", bufs=1)
                nc.tensor.matmul(pmu[:], s_ones[:], s4[:],
                                 start=True, stop=True)
                nc.tensor.matmul(psq[:], s_ones[:], ssq[:],
                                 start=True, stop=True)
                smusq = work.tile([1, C2], F32, tag="smusq")
                nc.scalar.activation(smusq[:], pmu[:], AF.Square)
                svar = work.tile([1, C2], F32, tag="svar")
                nc.vector.tensor_sub(svar[:], psq[:], smusq[:])
                # rsqrt(var+eps) = exp(-0.5 * ln(var+eps))
                slnv = work.tile([1, C2], F32, tag="slnv")
                nc.scalar.activation(slnv[:], svar[:], AF.Ln, bias=s_eps[:])
                srstd = work.tile([1, C2], F32R, tag="srstd")
                nc.scalar.activation(srstd[:], slnv[:], AF.Exp, scale=-0.5)
                srcp = work.tile([1, C2], F32R, tag="srcp")
                nc.scalar.activation(srcp[:], slnv[:], AF.Exp, scale=0.5)
                smu = work.tile([1, C2], F32R, tag="smu")
                nc.scalar.copy(smu[:], pmu[:])

                # B1 = broadcast rstd over 96 channel rows
                pB1 = psum.tile([DI, C2], F32, tag=f"m96_{fam}", bufs=1)
                nc.tensor.matmul(pB1[:], s_ones1[:], srstd[:],
                                 start=True, stop=True)
                sB1 = work.tile([DM, C2], BF16, tag="sB1")
                nc.scalar.copy(sB1[:], pB1[:])

                # pq = Wout·gamma @ s4 + vgneg (x) mu + vbeta (x) (1/rstd);
                # the final multiply by the broadcast rstd then yields
                # rstd*pq + vgneg*(mu*rstd) + vbeta.
                pq = psum.tile([DM, C2], F32, tag=f"pq_{fam}", bufs=1)
                nc.tensor.matmul(pq[:], s_wgT[:], s4[:],
                                 start=True, stop=False)
                nc.tensor.matmul(pq[:], s_vg1[:], smu[:],
                                 start=False, stop=False)
                nc.tensor.matmul(pq[:], s_vb1[:], srcp[:],
                                 start=False, stop=True)
                so = work.tile([DM, C2], F32, tag="so")
                nc.vector.tensor_mul(so[:], pq[:], sB1[:])
                nc.sync.dma_start(out2.ap()[:, sl], so[:])

    nc.compile()
    return nc


# ---------------------------------------------------------------------------
# host orchestration
# ---------------------------------------------------------------------------

_CACHE: dict = {}


def _programs():
    if "p1" not in _CACHE:
        _CACHE["p1"] = build_scan_program()
        _CACHE["p2"] = build_merge_program()
    return _CACHE["p1"], _CACHE["p2"]


def kernel(x, y, Wx, Wy, x_proj_weight, dt_projs_weight, dt_projs_bias,
           A_logs, Ds, ln_gamma, ln_beta, Wout):
    x = np.asarray(x, np.float32)
    y = np.asarray(y, np.float32)
    f8 = lambda a: np.asarray(a, np.float64)

    wsel_np, ysel_np = _selectors()
    pd, pn = _lane_maps()
    A = -np.exp(f8(A_logs)).reshape(K, DI, N)
    Dv = f8(Ds).reshape(K, DI)

    nc1, nc2 = _programs()

    in_maps1 = []
    for core in range(N_CORES):
        b, k = core // K, core % K
        Wd = (f8(dt_projs_weight)[k] @ f8(x_proj_weight)[k][:R] @ f8(Wx))
        WB = f8(x_proj_weight)[k][R:R + N] @ f8(Wx)     # [N, DM]
        WC = f8(x_proj_weight)[k][R + N:] @ f8(Wx)

        asc = np.empty((128, NT), np.float32)
        for t in range(NT):
            asc[:, t] = A[k][DPT * t + pd, pn]

        in_maps1.append(dict(
            xk=_perm(x[b], k),
            yk=_perm(y[b], k),
            wdT=np.ascontiguousarray(Wd.T.astype(np.float32)),
            wuT=np.ascontiguousarray(f8(Wy).T.astype(np.float32)),
            wBrepT=np.ascontiguousarray(WB[pn].T.astype(np.float32)),
            wCrepT=np.ascontiguousarray(WC[pn].T.astype(np.float32)),
            dtbias=np.asarray(dt_projs_bias, np.float32)[k].reshape(DI, 1),
            ascale=asc,
            wsel=wsel_np.astype(ml_dtypes.bfloat16),
            ysel=ysel_np.astype(ml_dtypes.bfloat16),
        ))

    res1 = bass_utils.run_bass_kernel_spmd(nc1, in_maps1,
                                           core_ids=list(range(N_CORES)))
    _CACHE["res1"] = res1

    # un-permute each direction's contribution back to row-major order
    contrib = np.empty((B, K, DI, L), np.float32)
    for core in range(N_CORES):
        b, k = core // K, core % K
        contrib[b, k] = _unperm(
            np.asarray(res1.results[core]["yc"]).astype(np.float32), k)

    wgT = np.ascontiguousarray(
        (f8(Wout) * f8(ln_gamma)[None, :]).T.astype(np.float32))
    vgneg = (-(f8(Wout) @ f8(ln_gamma))).astype(np.float32)
    vbeta = (f8(Wout) @ f8(ln_beta)).astype(np.float32)
    dvs = Dv.sum(axis=0).astype(np.float32).reshape(DI, 1)
    onesM = np.full((DI, 1), 1.0 / DI, np.float32)
    ones1 = np.ones((1, DI), np.float32)
    yrow = [np.ascontiguousarray(y[b].reshape(DM, L)).astype(ml_dtypes.bfloat16)
            for b in range(B)]

    in_maps2 = []
    for core in range(N_CORES):
        b, q = core // K, core % K
        sl = slice(q * L2, (q + 1) * L2)
        m = {f"c{i}": np.ascontiguousarray(contrib[b, i][:, sl])
             for i in range(K)}
        m.update(ykq=np.ascontiguousarray(yrow[b][:, sl]),
                 wgT=wgT, wuT=np.ascontiguousarray(f8(Wy).T.astype(ml_dtypes.bfloat16)),
                 vg1=vgneg.reshape(1, DM), vb1=vbeta.reshape(1, DM),
                 dvs=dvs, onesM=onesM, ones1=ones1,
                 epsv=np.full((1, 1), LN_EPS, np.float32))
        in_maps2.append(m)

    res2 = bass_utils.run_bass_kernel_spmd(nc2, in_maps2,
                                           core_ids=list(range(N_CORES)))
    _CACHE["res2"] = res2

    out = np.empty((B, DM, L), np.float32)
    for core in range(N_CORES):
        b, q = core // K, core % K
        out[b][:, q * L2:(q + 1) * L2] = res2.results[core]["out2"]
    return out.reshape(B, DM, H, W)
